# revision 27
# baseline (speedup 1.0000x reference)
"""GCNConv-style GNN layer on 8 Trainium2 NeuronCores (Bass/Tile).

Reference computation (B=8, N=4096, C=128, E=131072):
    adj  = symmetric 0/1 adjacency from edge_index, zero diagonal
    h    = x @ W0 + b0
    agg  = adj @ h            (per batch)
    out  = (cat[x, agg] @ W1 + b1) @ W2 + b2
    out  = gelu(out) @ Wo + bo
    ret  = x + out

Algebraic refactor (all linear maps before the single GELU compose; fold
them on the host at O(C^2) cost, and push the agg-path weight through
the aggregation since (adj @ x) @ Wa == adj @ (x @ Wa)):
    W12  = W1 @ W2                  [2C, C]
    Wx   = W12[:C]                  x-path weight
    Wa   = W0 @ W12[C:]             agg-path weight
    b0a  = b0 @ W12[C:]
    b12  = b1 @ W2 + b2
    y    = x @ Wa + 1⊗b0a           (host; deg⊗b0a == adj @ 1⊗b0a)
    pre  = adj @ y + x @ Wx + b12
    ret  = x + gelu(pre) @ Wo + bo

Device work per core (node partition, NS=512 rows each, SPMD, no
collectives): sT' = (adj @ y)^T is accumulated directly in PSUM by
making y tiles the stationary matmul operand and streaming this core's
adjacency columns as the moving operand; both operands are fp8 e4m3
(adjacency 0/1 exact; only y pays quantization — end-to-end rel err
1.1e-2 vs the 2e-2 gate).  Matmuls rotate across all 8 PSUM banks
every instruction (empirically ~1.4x faster than any same-bank
blocking), with bank finishes staggered a few slots apart so the 8
GELU activations (ACT, ~720 ns each) pipeline behind the remaining
matmul stream.  The x@Wx term is one bf16 matmul accumulated into the
same PSUM bank; gelu reads PSUM directly, the Wo matmul and the bf16
residual add + store trail per batch.  Host un-transposes during
unsharding.
"""

import numpy as np
import ml_dtypes

import bass_rust
import concourse.bass as bass
import concourse.mybir as mybir
import concourse.tile as tile
from concourse.bass_utils import run_bass_kernel_spmd

B, N, C, E = 8, 4096, 128, 131072
NCORES = 8
NS = N // NCORES          # 512 output rows per core
KC = N // 128             # 32 k-chunks over the contraction dim
COLS = B * C              # 1024 columns of y_r  (b-major, c-minor)
RCOLS = B * NS            # 4096 columns of transposed row-space tiles

F32 = mybir.dt.float32
BF16 = mybir.dt.bfloat16
FP8 = mybir.dt.float8e4
BF16_NP = ml_dtypes.bfloat16
FP8_NP = ml_dtypes.float8_e4m3


def _split_multiwaits(nc, max_waits=1):
    """Walrus (CoreV3) refuses instructions with more than one sync wait.
    Tile's tail drain can carry several; hoist the extras onto preceding
    single-wait EventSemaphore instructions on the same engine."""
    for blk in nc.m.functions[0].blocks:
        new_list = []
        for ins in blk.instructions:
            si = ins.sync_info
            if si is not None and si.on_wait and len(si.on_wait) > max_waits:
                waits = list(si.on_wait)
                extra, keep = waits[:-max_waits], waits[-max_waits:]
                for i, w in enumerate(extra):
                    ev = mybir.InstEventSemaphore(
                        name=f"{ins.name}_wsplit{i}",
                        engine=ins.engine,
                        ins=[],
                        outs=[],
                        sync_info=bass_rust.SyncInfo(on_wait=[w], on_update=[]),
                    )
                    new_list.append(ev)
                si.on_wait = keep
            new_list.append(ins)
        blk.instructions[:] = new_list


# per-bank start-pass deferral: staggers bank finishes ~one rotation
# pass apart so the ACT gelu stream pipelines behind the PE tail
STAG = (0, 1, 2, 3, 4, 5, 6, 7)


def build_bass(niter=1, stage="full", stag=STAG, lag=1, evac="act",
               out_mode="sliced"):
    nc = bass.Bass()

    yr_d = nc.dram_tensor("yr", [N, COLS], FP8, kind="ExternalInput")
    adjT_d = nc.dram_tensor("adjT", [N, NS], FP8, kind="ExternalInput")
    xt_bf_d = nc.dram_tensor("xt_bf", [C, RCOLS], BF16, kind="ExternalInput")
    xtbo_d = nc.dram_tensor("xtbo", [C, RCOLS], BF16, kind="ExternalInput")
    wx_d = nc.dram_tensor("wx", [C, C], BF16, kind="ExternalInput")
    wo_d = nc.dram_tensor("wo", [C, C], BF16, kind="ExternalInput")
    id_d = nc.dram_tensor("ident", [C, C], BF16, kind="ExternalInput")
    b12_d = nc.dram_tensor("b12", [C, 1], F32, kind="ExternalInput")
    out_d = nc.dram_tensor("out", [C, RCOLS], BF16, kind="ExternalOutput")

    with tile.TileContext(nc) as tc:
        with (
            tc.tile_pool(name="const", bufs=1) as const,
            tc.tile_pool(name="big", bufs=1) as big,
        ):

            def body(_iv=0):
                # ---- resident inputs: k-chunk streams first (the s-stage
                # matmuls chase them), then weights, then per-batch slices
                # of the MLP-side tensors in consumption order.
                yr_sb = big.tile([128, KC, COLS], FP8)
                adjT_sb = big.tile([128, KC, NS], FP8)
                xt_bf_sb = big.tile([C, RCOLS], BF16)
                xtbo_sb = big.tile([C, RCOLS], BF16)
                wx_sb = const.tile([C, C], BF16)
                wo_sb = const.tile([C, C], BF16)
                id_sb = const.tile([C, C], BF16)
                b12_sb = const.tile([C, 1], F32)
                for k in range(KC):
                    nc.sync.dma_start(out=adjT_sb[:, k, :], in_=adjT_d[k * 128:(k + 1) * 128, :])
                    nc.sync.dma_start(out=yr_sb[:, k, :], in_=yr_d[k * 128:(k + 1) * 128, :])
                nc.sync.dma_start(out=wx_sb[:], in_=wx_d[:])
                nc.sync.dma_start(out=wo_sb[:], in_=wo_d[:])
                nc.sync.dma_start(out=id_sb[:], in_=id_d[:])
                nc.sync.dma_start(out=b12_sb[:], in_=b12_d[:])
                for b in range(B):
                    cs = slice(b * NS, (b + 1) * NS)
                    nc.sync.dma_start(out=xt_bf_sb[:, cs], in_=xt_bf_d[:, cs])
                    nc.sync.dma_start(out=xtbo_sb[:, cs], in_=xtbo_d[:, cs])

                gelu_sb = big.tile([C, RCOLS], BF16)
                res_sb = big.tile([C, RCOLS], BF16)
                if evac == "act_dve":
                    tmp_sb = big.tile([C, RCOLS], BF16, name="tmp_sb")
                else:
                    tmp_sb = None
                with tc.tile_pool(name="psum", bufs=8, space="PSUM") as psum:
                    ps = [
                        psum.tile([128, NS], F32, tag="ps", name=f"acc_{bc}")
                        for bc in range(B)
                    ]

                    def po_chain(b):
                        # residual add rides the PE (identity-stationary
                        # accumulate); PSUM evac via ACT — DVE's PSUM port
                        # is ~7x slower and was the old kernel's bottleneck
                        cols = slice(b * NS, (b + 1) * NS)
                        po = psum.tile([128, NS], F32, tag="ps", name=f"out_{b}")
                        if evac == "act_dve":
                            # residual added by DVE in SBUF-land (its fast
                            # port); PE saves the identity matmul
                            nc.tensor.matmul(po, wo_sb[:], gelu_sb[:, cols],
                                             start=True, stop=True)
                            nc.scalar.activation(
                                out=tmp_sb[:, cols], in_=po[:],
                                func=mybir.ActivationFunctionType.Identity,
                            )
                            nc.vector.tensor_add(out=res_sb[:, cols],
                                                 in0=tmp_sb[:, cols],
                                                 in1=xtbo_sb[:, cols])
                            nc.sync.dma_start(out=out_d[:, cols],
                                              in_=res_sb[:, cols])
                            return
                        nc.tensor.matmul(po, wo_sb[:], gelu_sb[:, cols],
                                         start=True, stop=False)
                        nc.tensor.matmul(po, id_sb[:], xtbo_sb[:, cols],
                                         start=False, stop=True)
                        if evac == "act":
                            nc.scalar.activation(
                                out=res_sb[:, cols], in_=po[:],
                                func=mybir.ActivationFunctionType.Identity,
                            )
                            if out_mode == "sliced":
                                nc.sync.dma_start(out=out_d[:, cols],
                                                  in_=res_sb[:, cols])

                    # ---- s-stage: bank-rotating fp8 matmuls; bank bc's
                    # k-pass is deferred stag[bc] passes so finishes
                    # stagger.  wx/gelu/po chains interleave into the
                    # stream as each bank completes its contraction.
                    done = []
                    maxP = KC - 1 + max(stag)
                    for P in range(maxP + 1):
                        slots = [(P - stag[bc], bc) for bc in range(B)
                                 if 0 <= P - stag[bc] < KC]
                        for k, bc in sorted(slots):
                            nc.tensor.matmul(
                                ps[bc],
                                yr_sb[:, k, bc * 128:(bc + 1) * 128],
                                adjT_sb[:, k, :],
                                start=(k == 0),
                                stop=False,
                            )
                            if k == KC - 1 and stage in ("full", "gelu"):
                                cols = slice(bc * NS, (bc + 1) * NS)
                                nc.tensor.matmul(
                                    ps[bc], wx_sb[:], xt_bf_sb[:, cols],
                                    start=False, stop=True)
                                nc.scalar.activation(
                                    out=gelu_sb[:, cols], in_=ps[bc][:],
                                    func=mybir.ActivationFunctionType.Gelu,
                                    bias=b12_sb[:, 0:1], scale=1.0,
                                )
                                done.append(bc)
                                if stage == "full" and len(done) > lag:
                                    po_chain(done[-1 - lag])
                    if stage == "full":
                        for b in done[-lag:] if lag > 0 else []:
                            po_chain(b)
                        if evac == "act" and out_mode == "big":
                            nc.sync.dma_start(out=out_d[:], in_=res_sb[:])
                        elif evac == "none":
                            nc.sync.dma_start(out=out_d[:], in_=gelu_sb[:])
                    elif stage == "gelu":
                        nc.sync.dma_start(out=out_d[:], in_=gelu_sb[:])
                    else:
                        for bc in range(B):
                            nc.tensor.matmul(ps[bc], wx_sb[:],
                                             xt_bf_sb[:, bc * NS:(bc + 1) * NS],
                                             start=False, stop=True)
                            nc.vector.tensor_copy(
                                out=res_sb[:, bc * NS:(bc + 1) * NS],
                                in_=ps[bc])
                        nc.sync.dma_start(out=out_d[:], in_=res_sb[:])

            if niter == 1:
                body()
            else:
                with tc.For_i(0, niter, 1, hint_engines=(mybir.EngineType.PE,)):
                    body()

    _split_multiwaits(nc)
    return nc


def host_prep(x, edge_index, W0, b0, W1, b1, W2, b2, Wo, bo):
    """Fold weights, build the dense adjacency, lay out per-core inputs."""
    x = np.asarray(x, np.float32)
    ei = np.asarray(edge_index, np.int64)
    W0, b0, W1, b1, W2, b2, Wo, bo = (
        np.asarray(a, np.float32) for a in (W0, b0, W1, b1, W2, b2, Wo, bo)
    )

    # dense symmetric adjacency with set-semantics dedup, zero diagonal
    k1 = ei[0] * N + ei[1]
    k2 = ei[1] * N + ei[0]
    keys = np.unique(np.concatenate([k1, k2]))
    rows = keys // N
    cols = keys % N
    off_diag = rows != cols
    keys = keys[off_diag]
    adj = np.zeros(N * N, np.uint8)
    adj[keys] = 0x38  # fp8 e4m3 1.0 bit pattern
    adj = adj.reshape(N, N).view(FP8_NP)

    # folded weights
    W12 = W1 @ W2                      # [2C, C]
    Wx = W12[:C]
    W12a = W12[C:]
    Wa = W0 @ W12a
    b0a = b0 @ W12a                    # [C]
    b12 = (b1 @ W2 + b2).reshape(C, 1)

    y = x @ Wa + b0a[None, None, :]                               # [B,N,C]
    yr = np.ascontiguousarray(
        y.transpose(1, 0, 2).reshape(N, B * C)).astype(FP8_NP)    # [N,(b,c)]
    xt = x.transpose(2, 0, 1)                                     # [C,B,N] f32

    in_maps = []
    for c in range(NCORES):
        rs = slice(c * NS, (c + 1) * NS)
        xt_c = np.ascontiguousarray(xt[:, :, rs]).reshape(C, RCOLS)
        in_maps.append({
            "yr": yr,
            "adjT": np.ascontiguousarray(adj[:, rs]),
            "xt_bf": xt_c.astype(BF16_NP),
            "xtbo": (xt_c + bo[:, None]).astype(BF16_NP),
            "wx": Wx.astype(BF16_NP),
            "wo": Wo.astype(BF16_NP),
            "ident": np.eye(C, dtype=BF16_NP),
            "b12": b12,
        })
    return in_maps


def assemble_output(results):
    out = np.empty((B, N, C), np.float32)
    for c in range(NCORES):
        r = np.asarray(results[c]["out"], np.float32)  # [C, (b, row)] bf16
        out[:, c * NS:(c + 1) * NS, :] = r.reshape(C, B, NS).transpose(1, 2, 0)
    return out


_NC_CACHE = []


def kernel(x, edge_index, W0, b0, W1, b1, W2, b2, Wo, bo):
    in_maps = host_prep(x, edge_index, W0, b0, W1, b1, W2, b2, Wo, bo)
    if not _NC_CACHE:
        _NC_CACHE.append(build_bass())
    nc = _NC_CACHE[0]
    res = run_bass_kernel_spmd(nc, in_maps, list(range(NCORES)))
    return assemble_output(res.results)
